# revision 18
# baseline (speedup 1.0000x reference)
"""GIN message-passing GNN (nn_FNSD_51762945852046) on 8 Trainium2 NeuronCores.

Strategy
--------
Nodes are sharded 8 ways (12500 -> padded 12544 per core, degree-sorted
within each shard). Full node features h live replicated per core in HBM
([100352, 128], node-major) and are rebuilt each layer with an AllGather.

Per layer, per core:
  A) neighbor aggregation: per-edge features are fetched with int16
     dma_gather instructions (4 source windows of 25088 rows to fit int16
     indexing), 128-edge "columns" are reduced onto the 128 destinations
     of a node tile with selection matmuls (sel[p, i] = dest_local[p]==i)
     accumulating in PSUM; (tile, window) runs accumulate into an SBUF
     aggregation buffer.
  B) dense MLP in feature-major layout: z0 = u @ W1 (PE), training-mode
     BatchNorm stats via partial sums + a tiny AllReduce, fused
     scale/bias/ReLU on the scalar engine, z2 = zr @ W2, residual +
     LayerNorm back in node-major layout (bn_stats/bn_aggr per node).
Global mean pool: one-hot(graph) selection matmuls accumulate per-graph
sums in PSUM; host sums the 8 partial results and divides by counts.
"""
import numpy as np

import concourse.bass as bass
import concourse.bacc as bacc
import concourse.tile as tile
from concourse import mybir
from concourse import bass_utils
from concourse.masks import make_identity

F32 = mybir.dt.float32
I32 = mybir.dt.int32
I16 = mybir.dt.int16

P = 128
H = 128
L = 4
NG = 64
NC = 8
BN_EPS = 1e-5
LN_EPS = 1e-5


class Cfg:
    def __init__(self, n_nodes, n_shard):
        self.N = n_nodes              # real nodes
        self.NSH = n_shard            # real nodes per core
        self.NSHP = ((n_shard + P - 1) // P) * P  # padded per-core
        self.TILES = self.NSHP // P
        self.V = NC * self.NSHP       # padded total (hfull rows)
        assert self.V % 4 == 0
        self.WIN = self.V // 4        # int16-addressable window
        assert self.WIN <= 32767
        self.CH = 12                  # gather chunk (columns per dma_gather)
        self.MMCH = 512               # matmul chunk (nodes per PE matmul)


CFG_FULL = Cfg(100000, 12500)


# ---------------------------------------------------------------------------
# host-side preprocessing: sharding, degree sort, edge schedule
# ---------------------------------------------------------------------------

def preprocess(edge_index, batch, cfg):
    N, NSH, NSHP, TILES, V, WIN = (
        cfg.N, cfg.NSH, cfg.NSHP, cfg.TILES, cfg.V, cfg.WIN)
    row = np.asarray(edge_index[0], np.int64)
    col = np.asarray(edge_index[1], np.int64)
    r2 = np.concatenate([row, col])
    c2 = np.concatenate([col, row])
    keep = r2 != c2
    r2, c2 = r2[keep], c2[keep]

    deg = np.bincount(r2, minlength=N)

    store_of_orig = np.empty(N, np.int64)
    orig_of_store = np.full(V, -1, np.int64)
    for c in range(NC):
        orig = np.arange(c * NSH, (c + 1) * NSH)
        order = np.argsort(-deg[orig], kind="stable")
        so = orig[order]
        store = c * NSHP + np.arange(NSH)
        store_of_orig[so] = store
        orig_of_store[store] = so

    sdst = store_of_orig[r2]
    ssrc = store_of_orig[c2]
    ecore = sdst // NSHP
    ewin = ssrc // WIN
    sloc = sdst % NSHP
    etile = sloc // P
    edl = sloc % P

    # run = (window, tile); count per (core, win, tile)
    cnt = np.zeros((NC, 4, TILES), np.int64)
    np.add.at(cnt, (ecore, ewin, etile), 1)
    ncols = np.ceil(cnt.max(axis=0) / P).astype(np.int64)  # [4, TILES]

    # global column layout: window-major, then tile
    col_start = np.zeros((4, TILES), np.int64)
    acc = 0
    runs = []  # (w, t, c0, c1) in emission order
    for w in range(4):
        for t in range(TILES):
            nc_ = int(ncols[w, t])
            col_start[w, t] = acc
            if nc_ > 0:
                runs.append((w, t, acc, acc + nc_))
            acc += nc_
    NCOLS = acc

    # chunks: consecutive columns within one window
    chunks = []  # (w, c0, ncols_chunk)
    for w in range(4):
        base = int(col_start[w, 0] if TILES else 0)
        wcols = int(ncols[w].sum())
        done = 0
        while done < wcols:
            ch = min(cfg.CH, wcols - done)
            chunks.append((w, base + done, ch))
            done += ch

    # per-column tile id / run membership
    tile_of_col = np.full(NCOLS, -1, np.int64)
    for (w, t, c0, c1) in runs:
        tile_of_col[c0:c1] = t

    # per-tile run lists
    runs_of_tile = [[] for _ in range(TILES)]
    for ri, (w, t, c0, c1) in enumerate(runs):
        runs_of_tile[t].append(ri)

    # per-core slot data
    dloc_arr = np.full((NC, P, NCOLS), 255, np.int32)
    idxval = np.zeros((NC, NCOLS * P), np.int64)  # window-local src per slot
    order = np.lexsort((ssrc, etile, ewin, ecore))
    r2o = r2[order]
    sg = (ecore * 4 + ewin) * TILES + etile
    sgo = sg[order]
    # rank of each edge within its (core, win, tile) group
    grp_change = np.empty(len(sgo), bool)
    if len(sgo):
        grp_change[0] = True
        grp_change[1:] = sgo[1:] != sgo[:-1]
        gidx = np.cumsum(grp_change) - 1
        grp_first = np.where(grp_change)[0]
        rank = np.arange(len(sgo)) - grp_first[gidx]
        eco = ecore[order]
        ewo = ewin[order]
        eto = etile[order]
        ecol = col_start[ewo, eto] + rank // P
        ep = rank % P
        dloc_arr[eco, ep, ecol] = edl[order]
        idxval[eco, ecol * P + ep] = ssrc[order] % WIN

    # pack int16 index blocks per chunk
    blocks = []
    for (w, c0, ch) in chunks:
        v = idxval[:, c0 * P:(c0 + ch) * P]  # [NC, ch*128]
        assert v.max(initial=0) < 32768
        b = v.reshape(NC, ch * 8, 16).transpose(0, 2, 1)  # [NC, 16, ch*8]
        blocks.append(np.tile(b, (1, 8, 1)).astype(np.int16))
    idx16 = np.concatenate(blocks, axis=2) if blocks else np.zeros(
        (NC, 128, 0), np.int16)

    # pooling graph ids, node-major [P, TILES]
    batch = np.asarray(batch, np.int64)
    gid = np.full((NC, P, TILES), 255, np.int32)
    for c in range(NC):
        st = c * NSHP + np.arange(NSHP)
        og = orig_of_store[st]
        g = np.where(og >= 0, batch[np.clip(og, 0, N - 1)], 255)
        gid[c] = g.reshape(TILES, P).T

    sched = {
        "runs": runs,
        "runs_of_tile": runs_of_tile,
        "chunks": chunks,
        "tile_of_col": tile_of_col,
        "NCOLS": NCOLS,
        "IDXW": idx16.shape[2],
    }
    return sched, {
        "idx16": idx16, "dloc": dloc_arr, "gid": gid,
        "store_of_orig": store_of_orig, "orig_of_store": orig_of_store,
    }


# ---------------------------------------------------------------------------
# device program
# ---------------------------------------------------------------------------

def build_nc(cfg, sched):
    import os
    n_layers = int(os.environ.get("K_LAYERS", str(L)))
    phase = os.environ.get("K_PHASE", "full")  # full|enc|noagg|nomlp|nocc
    do_agg = phase not in ("enc", "noagg")
    do_mlp = phase not in ("enc", "nomlp")
    do_cc = phase != "nocc" and os.environ.get("K_CC", "1") == "1"
    # aggregation sub-phase: full|gather|gathersel|nogather
    agg_mode = os.environ.get("K_AGG", "full")
    ag_gather = do_agg and agg_mode in ("full", "gather", "gathersel")
    ag_sel = do_agg and agg_mode in ("full", "gathersel", "nogather")
    ag_mm = do_agg and agg_mode in ("full", "nogather")
    if phase == "enc":
        n_layers = 0
    NSHP, TILES, V, WIN = cfg.NSHP, cfg.TILES, cfg.V, cfg.WIN
    NCOLS, IDXW = sched["NCOLS"], sched["IDXW"]
    runs = sched["runs"]
    runs_of_tile = sched["runs_of_tile"]
    chunks = sched["chunks"]
    MM = cfg.MMCH
    mm_chunks = []
    off = 0
    while off < NSHP:
        w = min(MM, NSHP - off)
        mm_chunks.append((off, w))
        off += w

    nc = bacc.Bacc("TRN2", target_bir_lowering=False, debug=False,
                   num_devices=NC, num_swdge_queues=4)

    # inputs
    xT_d = nc.dram_tensor("xT", [P, NSHP], F32, kind="ExternalInput")
    idx_d = nc.dram_tensor("idx16", [P, max(IDXW, 8)], I16, kind="ExternalInput")
    dlc_d = nc.dram_tensor("dloc", [P, max(NCOLS, 1)], I32, kind="ExternalInput")
    gid_d = nc.dram_tensor("gid", [P, TILES], I32, kind="ExternalInput")
    w0_d = nc.dram_tensor("w0", [H, H], F32, kind="ExternalInput")
    b0_d = nc.dram_tensor("b0", [H], F32, kind="ExternalInput")
    w1_d = nc.dram_tensor("w1", [L, H, H], F32, kind="ExternalInput")
    w2_d = nc.dram_tensor("w2", [L, H, H], F32, kind="ExternalInput")
    b2_d = nc.dram_tensor("b2", [L, H], F32, kind="ExternalInput")
    bng_d = nc.dram_tensor("bng", [L, H], F32, kind="ExternalInput")
    bnb_d = nc.dram_tensor("bnb", [L, H], F32, kind="ExternalInput")
    lng_d = nc.dram_tensor("lng", [L, H], F32, kind="ExternalInput")
    lnb_d = nc.dram_tensor("lnb", [L, H], F32, kind="ExternalInput")
    eps_d = nc.dram_tensor("eps1p", [L], F32, kind="ExternalInput")
    pool_d = nc.dram_tensor("pool", [NG, H], F32, kind="ExternalOutput")

    # internal DRAM
    hfull = nc.dram_tensor("hfull", [V, H], F32,
                           addr_space="Shared" if do_cc else "Local")
    hsh = nc.dram_tensor("hsh", [NSHP, H], F32)
    st_in = nc.dram_tensor("st_in", [P, 2], F32)
    st_out = nc.dram_tensor("st_out", [P, 2], F32,
                            addr_space="Shared" if do_cc else "Local")

    groups = [list(range(NC))]

    with tile.TileContext(nc) as tc:
        import contextlib
        with contextlib.ExitStack() as ctx:
            persist = ctx.enter_context(tc.tile_pool(name="persist", bufs=1))
            consts = ctx.enter_context(tc.tile_pool(name="consts", bufs=2))
            chk = ctx.enter_context(tc.tile_pool(name="chk", bufs=3))
            stage = ctx.enter_context(tc.tile_pool(name="stage", bufs=3))
            small = ctx.enter_context(tc.tile_pool(name="small", bufs=2))
            psA = ctx.enter_context(tc.tile_pool(name="psA", bufs=4, space="PSUM"))
            psT = ctx.enter_context(tc.tile_pool(name="psT", bufs=3, space="PSUM"))
            psM = ctx.enter_context(tc.tile_pool(name="psM", bufs=1, space="PSUM"))

            h_nm = persist.tile([P, NSHP], F32, tag="h_nm")
            bufA = persist.tile([P, NSHP], F32, tag="bufA")  # agg / z0
            bufU = persist.tile([P, NSHP], F32, tag="bufU")  # xT / u / zr
            iota = persist.tile([P, P], I32, tag="iota")
            ident = persist.tile([P, P], F32, tag="ident")
            gidt = persist.tile([P, TILES], I32, tag="gidt")

            nc.gpsimd.iota(iota[:], pattern=[[1, P]], base=0,
                           channel_multiplier=0)
            make_identity(nc, ident[:])
            nc.sync.dma_start(out=gidt[:], in_=gid_d[:, :])

            def produce_h(i, src_fm_of_tile):
                """src_fm_of_tile(t) -> feature-major [P, P] AP for tile t.
                Transpose to node-major, residual+LN for layers, write h_nm."""
                pass  # unused; logic inlined below

            # ---------------- encoder ----------------
            nc.sync.dma_start(out=bufU[:], in_=xT_d[:, :])
            w0sb = consts.tile([H, H], F32, tag="wenc")
            nc.sync.dma_start(out=w0sb[:], in_=w0_d[:, :])
            b0c = consts.tile([P, 1], F32, tag="vec_b0")
            nc.sync.dma_start(out=b0c[:], in_=b0_d[:, None])

            for (off_, wid) in mm_chunks:
                pm = psM.tile([P, MM], F32, tag="mm", space="PSUM")
                nc.tensor.matmul(out=pm[:, :wid], lhsT=w0sb[:],
                                 rhs=bufU[:, off_:off_ + wid],
                                 start=True, stop=True)
                # h_fm = z + b0
                nc.vector.tensor_scalar(
                    out=bufA[:, off_:off_ + wid], in0=pm[:, :wid],
                    scalar1=b0c[:, :1], scalar2=None,
                    op0=mybir.AluOpType.add)
            for t in range(TILES):
                ts = slice(t * P, (t + 1) * P)
                pT = psT.tile([P, P], F32, tag="tr", space="PSUM")
                nc.tensor.transpose(out=pT[:], in_=bufA[:, ts],
                                    identity=ident[:])
                nc.vector.tensor_copy(out=h_nm[:, ts], in_=pT[:])
            for t_ in range(TILES):
                nc.sync.dma_start(
                    out=hsh[t_ * P:(t_ + 1) * P, :],
                    in_=h_nm[:, t_ * P:(t_ + 1) * P])
            if do_cc:
                nc.gpsimd.collective_compute(
                    "AllGather", mybir.AluOpType.bypass, replica_groups=groups,
                    ins=[hsh[:].opt()], outs=[hfull[:].opt()])
            else:
                nc.sync.dma_start(out=hfull[0:NSHP, :], in_=hsh[:, :])

            # ---------------- layers ----------------
            for li in range(n_layers):
                w1sb = consts.tile([H, H], F32, tag="w1sb")
                nc.sync.dma_start(out=w1sb[:], in_=w1_d[li, :, :])
                w2sb = consts.tile([H, H], F32, tag="w2sb")
                nc.sync.dma_start(out=w2sb[:], in_=w2_d[li, :, :])
                b2c = consts.tile([P, 1], F32, tag="vec_b2")
                nc.sync.dma_start(out=b2c[:], in_=b2_d[li, :, None])
                bngc = consts.tile([P, 1], F32, tag="vec_bng")
                nc.sync.dma_start(out=bngc[:], in_=bng_d[li, :, None])
                bnbc = consts.tile([P, 1], F32, tag="vec_bnb")
                nc.sync.dma_start(out=bnbc[:], in_=bnb_d[li, :, None])
                lngb = consts.tile([P, P], F32, tag="lngb")
                nc.sync.dma_start(out=lngb[:],
                                  in_=lng_d[li:li + 1, :].to_broadcast([P, H]))
                lnbb = consts.tile([P, P], F32, tag="lnbb")
                nc.sync.dma_start(out=lnbb[:],
                                  in_=lnb_d[li:li + 1, :].to_broadcast([P, H]))
                epsc = consts.tile([P, 1], F32, tag="vec_eps")
                nc.sync.dma_start(out=epsc[:],
                                  in_=eps_d[li:li + 1].to_broadcast([P, 1]))

                # --- A: aggregation ---
                if not do_agg:
                    for t in range(TILES):
                        nc.vector.memset(bufA[:, t * P:(t + 1) * P], 0.0)
                        emit_u_noagg = None
                run_psum = {}
                runs_done = [0] * TILES

                def emit_u(t):
                    ts = slice(t * P, (t + 1) * P)
                    unm = stage.tile([P, P], F32, tag="unm")
                    nc.vector.tensor_scalar(
                        out=unm[:], in0=h_nm[:, ts], scalar1=epsc[:, :1],
                        scalar2=None, op0=mybir.AluOpType.mult)
                    nc.vector.tensor_add(out=unm[:], in0=unm[:],
                                         in1=bufA[:, ts])
                    pT = psT.tile([P, P], F32, tag="tr", space="PSUM")
                    nc.tensor.transpose(out=pT[:], in_=unm[:],
                                        identity=ident[:])
                    nc.vector.tensor_copy(out=bufU[:, ts], in_=pT[:])

                idx_off = 0
                gq = 0
                for (w, c0, chw) in (chunks if do_agg else []):
                    gt = chk.tile([P, cfg.CH, P], F32, tag="gt")
                    if ag_mm and not ag_gather:
                        nc.vector.memset(gt[:, :chw, :], 0.0)
                    if ag_gather:
                        idxt = chk.tile([P, cfg.CH * 8], I16, tag="idx")
                        nc.sync.dma_start(
                            out=idxt[:, :chw * 8],
                            in_=idx_d[:, idx_off:idx_off + chw * 8])
                        nc.gpsimd.dma_gather(
                            gt[:, :chw, :], hfull[w * WIN:(w + 1) * WIN, :],
                            idxt[:, :chw * 8], chw * P, chw * P, P,
                            elem_step=P, single_packet=False, queue_num=gq)
                        gq = (gq + 1) % 4
                    selt = chk.tile([P, cfg.CH, P], F32, tag="sel")
                    if ag_sel:
                        dlt = chk.tile([P, cfg.CH], I32, tag="dl")
                        nc.sync.dma_start(out=dlt[:, :chw],
                                          in_=dlc_d[:, c0:c0 + chw])
                        nc.vector.tensor_tensor(
                            out=selt[:, :chw, :],
                            in0=dlt[:, :chw, None].to_broadcast([P, chw, P]),
                            in1=iota[:, None, :].to_broadcast([P, chw, P]),
                            op=mybir.AluOpType.is_equal)
                    for cl in (range(chw) if ag_mm else []):
                        c = c0 + cl
                        t = int(sched["tile_of_col"][c])
                        ri = None
                        for r_ in runs_of_tile[t]:
                            if runs[r_][2] <= c < runs[r_][3]:
                                ri = r_
                                break
                        _, _, rc0, rc1 = runs[ri]
                        if c == rc0:
                            run_psum[ri] = psA.tile([P, P], F32, tag="agg",
                                                    space="PSUM",
                                                    name="aggps")
                        nc.tensor.matmul(
                            out=run_psum[ri][:], lhsT=selt[:, cl, :],
                            rhs=gt[:, cl, :], start=(c == rc0),
                            stop=(c == rc1 - 1))
                        if c == rc1 - 1:
                            ts = slice(t * P, (t + 1) * P)
                            if runs_done[t] == 0:
                                nc.vector.tensor_copy(out=bufA[:, ts],
                                                      in_=run_psum[ri][:])
                            else:
                                nc.vector.tensor_add(
                                    out=bufA[:, ts], in0=bufA[:, ts],
                                    in1=run_psum[ri][:])
                            runs_done[t] += 1
                            del run_psum[ri]
                            if runs_done[t] == len(runs_of_tile[t]):
                                emit_u(t)
                    idx_off += chw * 8
                for t in range(TILES):
                    if not (runs_of_tile[t] and ag_mm):
                        nc.vector.memset(bufA[:, t * P:(t + 1) * P], 0.0)
                        emit_u(t)
                if cfg.NSH < NSHP:
                    nc.vector.memset(bufU[:, cfg.NSH:NSHP], 0.0)

                # --- B: MLP ---
                nmm = len(mm_chunks)
                if not do_mlp:
                    for t in range(TILES):
                        ts = slice(t * P, (t + 1) * P)
                        nc.vector.tensor_copy(out=h_nm[:, ts], in_=bufA[:, ts])
                if do_mlp:
                    sums = small.tile([P, nmm], F32, tag="sums")
                    sqs = small.tile([P, nmm], F32, tag="sqs")
                    for ci, (off_, wid) in enumerate(mm_chunks):
                        pm = psM.tile([P, MM], F32, tag="mm", space="PSUM")
                        nc.tensor.matmul(out=pm[:, :wid], lhsT=w1sb[:],
                                         rhs=bufU[:, off_:off_ + wid],
                                         start=True, stop=True)
                        nc.vector.tensor_copy(out=bufA[:, off_:off_ + wid],
                                              in_=pm[:, :wid])
                        nc.vector.reduce_sum(out=sums[:, ci:ci + 1],
                                             in_=pm[:, :wid],
                                             axis=mybir.AxisListType.X)
                        sq = stage.tile([P, MM], F32, tag="sq")
                        nc.scalar.activation(
                            out=sq[:, :wid], in_=pm[:, :wid],
                            func=mybir.ActivationFunctionType.Square,
                            accum_out=sqs[:, ci:ci + 1])
                    st = small.tile([P, 2], F32, tag="st")
                    nc.vector.reduce_sum(out=st[:, 0:1], in_=sums[:],
                                         axis=mybir.AxisListType.X)
                    nc.vector.reduce_sum(out=st[:, 1:2], in_=sqs[:],
                                         axis=mybir.AxisListType.X)
                    nc.sync.dma_start(out=st_in[:, :], in_=st[:])
                    if do_cc:
                        nc.gpsimd.collective_compute(
                            "AllReduce", mybir.AluOpType.add,
                            replica_groups=groups,
                            ins=[st_in[:].opt()], outs=[st_out[:].opt()])
                    else:
                        nc.sync.dma_start(out=st_out[:, :], in_=st_in[:, :])
                    stf = small.tile([P, 2], F32, tag="stf")
                    nc.sync.dma_start(out=stf[:], in_=st_out[:, :])

                    mean = small.tile([P, 1], F32, tag="mean")
                    nc.vector.tensor_scalar_mul(out=mean[:], in0=stf[:, 0:1],
                                                scalar1=1.0 / cfg.N)
                    msq = small.tile([P, 1], F32, tag="msq")
                    nc.vector.tensor_scalar_mul(out=msq[:], in0=stf[:, 1:2],
                                                scalar1=1.0 / cfg.N)
                    var = small.tile([P, 1], F32, tag="var")
                    nc.vector.tensor_mul(out=var[:], in0=mean[:], in1=mean[:])
                    nc.vector.tensor_tensor(out=var[:], in0=msq[:], in1=var[:],
                                            op=mybir.AluOpType.subtract)
                    epsbn = small.tile([P, 1], F32, tag="epsbn")
                    nc.vector.memset(epsbn[:], BN_EPS)
                    sd = small.tile([P, 1], F32, tag="sd")
                    nc.scalar.activation(out=sd[:], in_=var[:],
                                         func=mybir.ActivationFunctionType.Sqrt,
                                         bias=epsbn[:, :1])
                    rstd = small.tile([P, 1], F32, tag="rstd")
                    nc.vector.reciprocal(out=rstd[:], in_=sd[:])
                    sprime = small.tile([P, 1], F32, tag="sprime")
                    nc.vector.tensor_mul(out=sprime[:], in0=rstd[:], in1=bngc[:])
                    tprime = small.tile([P, 1], F32, tag="tprime")
                    nc.vector.tensor_mul(out=tprime[:], in0=mean[:], in1=sprime[:])
                    nc.vector.tensor_tensor(out=tprime[:], in0=bnbc[:],
                                            in1=tprime[:],
                                            op=mybir.AluOpType.subtract)

                    for (off_, wid) in mm_chunks:
                        nc.scalar.activation(
                            out=bufU[:, off_:off_ + wid],
                            in_=bufA[:, off_:off_ + wid],
                            func=mybir.ActivationFunctionType.Relu,
                            bias=tprime[:, :1], scale=sprime[:, :1])

                    epsln = small.tile([P, 1], F32, tag="epsln")
                    nc.vector.memset(epsln[:], LN_EPS)
                    for (off_, wid) in mm_chunks:
                        pm = psM.tile([P, MM], F32, tag="mm", space="PSUM")
                        nc.tensor.matmul(out=pm[:, :wid], lhsT=w2sb[:],
                                         rhs=bufU[:, off_:off_ + wid],
                                         start=True, stop=True)
                        z2r = stage.tile([P, MM], F32, tag="z2r")
                        nc.scalar.activation(
                            out=z2r[:, :wid], in_=pm[:, :wid],
                            func=mybir.ActivationFunctionType.Relu,
                            bias=b2c[:, :1])
                        for sub in range(wid // P):
                            t = (off_ + sub * P) // P
                            ts = slice(t * P, (t + 1) * P)
                            pT = psT.tile([P, P], F32, tag="tr", space="PSUM")
                            nc.tensor.transpose(out=pT[:],
                                                in_=z2r[:, sub * P:(sub + 1) * P],
                                                identity=ident[:])
                            nc.vector.tensor_add(out=h_nm[:, ts],
                                                 in0=h_nm[:, ts], in1=pT[:])
                            st6 = stage.tile([P, 6], F32, tag="st6")
                            nc.vector.bn_stats(out=st6[:], in_=h_nm[:, ts])
                            mv = stage.tile([P, 2], F32, tag="mv")
                            nc.vector.bn_aggr(out=mv[:], in_=st6[:])
                            sdl = stage.tile([P, 1], F32, tag="sdl")
                            nc.scalar.activation(
                                out=sdl[:], in_=mv[:, 1:2],
                                func=mybir.ActivationFunctionType.Sqrt,
                                bias=epsln[:, :1])
                            rsl = stage.tile([P, 1], F32, tag="rsl")
                            nc.vector.reciprocal(out=rsl[:], in_=sdl[:])
                            nc.vector.tensor_scalar(
                                out=h_nm[:, ts], in0=h_nm[:, ts],
                                scalar1=mv[:, 0:1], scalar2=rsl[:, :1],
                                op0=mybir.AluOpType.subtract,
                                op1=mybir.AluOpType.mult)
                            nc.vector.tensor_tensor(out=h_nm[:, ts],
                                                    in0=h_nm[:, ts], in1=lngb[:],
                                                    op=mybir.AluOpType.mult)
                            nc.vector.tensor_tensor(out=h_nm[:, ts],
                                                    in0=h_nm[:, ts], in1=lnbb[:],
                                                    op=mybir.AluOpType.add)

                if li < L - 1:
                    for t_ in range(TILES):
                        nc.sync.dma_start(
                            out=hsh[t_ * P:(t_ + 1) * P, :],
                            in_=h_nm[:, t_ * P:(t_ + 1) * P])
                    if do_cc:
                        nc.gpsimd.collective_compute(
                            "AllGather", mybir.AluOpType.bypass,
                            replica_groups=groups,
                            ins=[hsh[:].opt()], outs=[hfull[:].opt()])
                    else:
                        nc.sync.dma_start(out=hfull[0:NSHP, :], in_=hsh[:, :])

            # ---------------- global mean pool (partial sums) ----------------
            pp = psA.tile([P, P], F32, tag="agg", space="PSUM")
            for t in range(TILES):
                oh = stage.tile([P, NG], F32, tag="oh")
                nc.vector.tensor_tensor(
                    out=oh[:],
                    in0=gidt[:, t:t + 1].to_broadcast([P, NG]),
                    in1=iota[:, :NG],
                    op=mybir.AluOpType.is_equal)
                nc.tensor.matmul(out=pp[:NG, :], lhsT=oh[:],
                                 rhs=h_nm[:, t * P:(t + 1) * P],
                                 start=(t == 0), stop=(t == TILES - 1))
            po = small.tile([NG, H], F32, tag="po")
            nc.vector.tensor_copy(out=po[:], in_=pp[:NG, :])
            nc.sync.dma_start(out=pool_d[:, :], in_=po[:])

    nc.compile()
    return nc


# ---------------------------------------------------------------------------
# public entry point
# ---------------------------------------------------------------------------

def make_in_maps(inputs, cfg, sched, arrs):
    x = np.asarray(inputs["x"], np.float32)
    eps1p = (1.0 + np.asarray(inputs["eps_l"], np.float32)).astype(np.float32)
    in_maps = []
    oos = arrs["orig_of_store"]
    for c in range(NC):
        st = c * cfg.NSHP + np.arange(cfg.NSHP)
        og = oos[st]
        xs = np.zeros((cfg.NSHP, H), np.float32)
        real = og >= 0
        xs[real] = x[og[real]]
        m = {
            "xT": np.ascontiguousarray(xs.T),
            "idx16": np.ascontiguousarray(arrs["idx16"][c]) if arrs["idx16"].shape[2] else np.zeros((P, 8), np.int16),
            "dloc": np.ascontiguousarray(arrs["dloc"][c]) if sched["NCOLS"] else np.zeros((P, 1), np.int32),
            "gid": np.ascontiguousarray(arrs["gid"][c]),
            "w0": np.asarray(inputs["W0"], np.float32),
            "b0": np.asarray(inputs["b0"], np.float32),
            "w1": np.asarray(inputs["W1"], np.float32),
            "w2": np.asarray(inputs["W2"], np.float32),
            "b2": np.asarray(inputs["b2"], np.float32),
            "bng": np.asarray(inputs["bn_g"], np.float32),
            "bnb": np.asarray(inputs["bn_b"], np.float32),
            "lng": np.asarray(inputs["ln_g"], np.float32),
            "lnb": np.asarray(inputs["ln_b"], np.float32),
            "eps1p": eps1p,
        }
        in_maps.append(m)
    return in_maps


def finish_output(results, inputs, cfg):
    batch = np.asarray(inputs["batch"], np.int64)
    total = np.zeros((NG, H), np.float64)
    for c in range(NC):
        total += results[c]["pool"].astype(np.float64)
    cnt = np.bincount(batch, minlength=NG).astype(np.float64)
    out = total / np.maximum(cnt, 1.0)[:, None]
    return out.astype(np.float32)


_CACHE = {}


def _get_compiled(inputs, cfg=CFG_FULL):
    ei = np.asarray(inputs["edge_index"])
    key = hash(ei.tobytes()) ^ hash(np.asarray(inputs["batch"]).tobytes())
    if key not in _CACHE:
        sched, arrs = preprocess(ei, inputs["batch"], cfg)
        ncobj = build_nc(cfg, sched)
        _CACHE[key] = (ncobj, sched, arrs)
    return _CACHE[key]


def kernel(**inputs):
    cfg = CFG_FULL
    ncobj, sched, arrs = _get_compiled(inputs, cfg)
    in_maps = make_in_maps(inputs, cfg, sched, arrs)
    res = bass_utils.run_bass_kernel_spmd(
        ncobj, in_maps=in_maps, core_ids=list(range(NC)))
    return finish_output(res.results, inputs, cfg)

